# revision 12
# baseline (speedup 1.0000x reference)
"""LocalAttentionBlock on 8 trn2 cores.

Sharding: 8 cores = 2 batches x 4 sequence blocks of 512 queries.
Each core gets a zero-padded 1024-wide context window (block +/- 256),
transposed on host to [D, s] layout.  All matmuls in bf16 (f32 psum
accumulate).

Per-core pipeline (S^T layout: s on partitions, q on free dim):
  Head PAIRS (2m, 2m+1) are processed together: the S matmuls have
  K=64 contraction, so the even head runs on PE row-group 0 (k from
  kv_sb[0:64], q from qT[m][0:64]) and the odd head on row-group 64
  (khi/qT[64:128]) -- tile_position is auto-derived from the base
  partitions and the two matmuls execute CONCURRENTLY on the array.
  S^T band pieces for a pair land in five [128, 2, 512] psum chunk
  tiles; exp runs once per chunk over both heads (1024 cols), the
  |i-j|<=256 diagonal cut is 8 gpsimd affine_selects per PAIR (2D
  pattern spanning both heads).  The kv projection and q-projection
  accumulators live in the SAME psum ring as the S chunks (one
  [128,2,512] tile = two banks), so the attention middle starts as
  soon as kv + q(0..1) are done while the remaining q chains
  interleave into the first three attention iterations.  v_aug tiles
  ([v | valid] / [valid | v]) make the AV matmul emit numerator +
  64-way-replicated denominator in one pass; normalization:
  reciprocal_approx_fast + one DMA lane shift, per-pair anrm tiles so
  no consumer over-waits.  Wf: ft 0-3 chains interleave into the
  middle (partials in per-o SBUF tiles), ft 4-6 pre-run at the tail
  with ft 7 + fused bias/partial add (scalar_tensor_tensor) after the
  last norm; output DMAs alternate the scalar/sync queues.
"""
import sys

import ml_dtypes
import numpy as np

sys.path.insert(0, "/opt/trn_rl_repo")

import concourse.bass as bass  # noqa: E402,F401
import concourse.mybir as mybir  # noqa: E402
import concourse.tile as tile  # noqa: E402
from concourse import bacc  # noqa: E402
from concourse.bass import ts  # noqa: E402
from concourse.bass_utils import run_bass_kernel_spmd  # noqa: E402
from concourse.masks import make_identity  # noqa: E402

F32 = mybir.dt.float32
BF16 = mybir.dt.bfloat16
AF = mybir.ActivationFunctionType
ALU = mybir.AluOpType
BF = ml_dtypes.bfloat16

B, T, D = 2, 2048, 1024
NH, HD = 16, 64
WIN = 256
BLK = 512      # queries per core
CTX = 1024     # padded context width
NCORES = 8

# Per-pair S^T band chunks.  Each chunk is a [128, 2, 512] psum tile
# (even head bank | odd head bank).  pieces: (jt, qlo, off, w, start):
# s-tile jt covers s in [128jt, 128jt+128); piece covers q/cols
# [qlo, qlo+w) -> chunk cols [off, off+w).  start=True zeroes the bank.
# masks: (off, lo) -> 128-wide diagonal cut at chunk col off; lo keeps
# p - y >= 0, hi keeps y - p >= 0 (y local to the window).
CHUNKS = [
    ([(3, 0, 0, 512, True)],
     [(384, True)]),
    ([(4, 0, 0, 512, True)],
     [(0, False)]),
    ([(1, 0, 0, 256, True), (6, 256, 256, 256, False)],
     [(128, True), (256, False)]),
    ([(2, 0, 0, 384, True), (5, 384, 384, 128, False)],
     [(256, True)]),
    ([(0, 0, 0, 128, True), (7, 384, 128, 128, False),
      (5, 128, 256, 256, False)],
     [(0, True), (128, False), (256, False)]),
]


def _build():
    nc = bacc.Bacc(None)
    # weights come in pre-swizzled to the SBUF layout so every DMA is a
    # contiguous 2KB-per-partition transfer (strided rearrange DMAs cost
    # ~5us each): wq/wf rows are [p, m|o, dt|ft, c], wkv rows [p, dt, f]
    xT = nc.dram_tensor("xT", [D, CTX], BF16, kind="ExternalInput")
    wqT = nc.dram_tensor("wqT", [128, 8 * D], BF16, kind="ExternalInput")
    wkvT = nc.dram_tensor("wkvT", [128, 8 * 128], BF16, kind="ExternalInput")
    wfT = nc.dram_tensor("wfT", [128, 8 * D], BF16, kind="ExternalInput")
    bfin = nc.dram_tensor("bfin", [D, 1], F32, kind="ExternalInput")
    valid = nc.dram_tensor("valid", [128, 8], F32, kind="ExternalInput")
    yT = nc.dram_tensor("yT", [D, BLK], BF16, kind="ExternalOutput")

    with tile.TileContext(nc) as tc:
        with (
            tc.tile_pool(name="big", bufs=1) as big,
            tc.tile_pool(name="sm", bufs=1) as sm,
            tc.tile_pool(name="recp", bufs=4) as recp,
            tc.tile_pool(name="expp", bufs=10) as expp,
            tc.tile_pool(name="psM", bufs=2, space="PSUM") as psM,
            tc.tile_pool(name="psO", bufs=2, space="PSUM") as psO,
        ):
            # ---- input DMAs: one sync HWDGE ring, strict order =
            # stream priority.  xt first, wq0 right behind it (gates
            # the first S pair), then wq, then wf (needed mid-kernel).
            xt = big.tile([128, 8, CTX], BF16, tag="xt")
            wkv = big.tile([128, 8, 128], BF16, tag="wkv")
            wq = big.tile([128, 8, 8, 128], BF16, tag="wq")
            wf = big.tile([128, 8, 8, 128], BF16, tag="wf")
            bf_sb = sm.tile([128, 8], F32, tag="bf")
            valid_sb = sm.tile([128, 8], F32, tag="valid")
            nc.sync.dma_start(out=wkv[:, 0, :], in_=wkvT[:, 0:128])
            nc.sync.dma_start(out=xt[:, 0, :], in_=xT[ts(0, 128), :])
            nc.sync.dma_start(out=wkv[:, 1:8, :], in_=wkvT[:, 128:1024])
            nc.sync.dma_start(out=wq[:, 0, :, :], in_=wqT[:, 0:1024])
            for dt in range(1, 8):
                nc.sync.dma_start(out=xt[:, dt, :], in_=xT[ts(dt, 128), :])
            for m in range(1, 8):
                nc.sync.dma_start(out=wq[:, m, :, :], in_=wqT[:, ts(m, 1024)])
            nc.scalar.dma_start(
                out=bf_sb, in_=bfin.rearrange("(o p) x -> p (o x)", p=128))
            nc.scalar.dma_start(out=valid_sb, in_=valid[:, :])
            for o in range(0, 8, 4):
                nc.sync.dma_start(out=wf[:, o:o + 4, :, :],
                                  in_=wfT[:, o * 1024:(o + 4) * 1024])
            ident = sm.tile([128, 128], BF16, tag="ident")
            make_identity(nc, ident)
            ones64 = sm.tile([128, 64], BF16, tag="ones64")
            nc.vector.memset(ones64, 1.0)

            # kv_sb: rows 0:64 = k^T, rows 64:128 = v^T; khi rows 64:128
            # carry the same k^T so odd heads matmul from base 64.
            kv_sb = big.tile([128, CTX], BF16, tag="kv")
            khi = big.tile([128, CTX], BF16, tag="khi")
            vaug_e = big.tile([128, 8, 128], BF16, tag="vaug_e")
            vaug_o = big.tile([128, 8, 128], BF16, tag="vaug_o")
            qT = [big.tile([128, BLK], BF16, tag=f"qT{m}",
                           name=f"qT{m}") for m in range(8)]
            # per-pair normalized-attention tiles (no whole-tile WAR)
            anrmP = [big.tile([128, BLK], BF16, tag=f"an{m}",
                              name=f"anrm{m}") for m in range(8)]
            # per-o Wf stage-1 partials
            y1_sb = [big.tile([128, BLK], F32, tag=f"wfp{o}",
                              name=f"y1sb{o}") for o in range(8)]

            def s_tile(name):
                return psM.tile([128, 2, 512], F32, tag="S", name=name)

            # ---- kv projection in the first S-ring tile ----
            kv_ps = s_tile("kv_ps")
            for dt in range(8):
                for ch in range(2):
                    nc.tensor.matmul(kv_ps[:, ch, :], wkv[:, dt, :],
                                     xt[:, dt, ts(ch, 512)],
                                     start=(dt == 0), stop=(dt == 7))
            for ch in range(2):
                nc.scalar.activation(out=kv_sb[:, ts(ch, 512)],
                                     in_=kv_ps[:, ch, :], func=AF.Copy)
            nc.sync.dma_start(out=khi[64:128, :], in_=kv_sb[0:64, :])

            def emit_qpair(j):
                """q chains for q-tiles 2j and 2j+1 in one S-ring tile."""
                qp = s_tile(f"qp{j}")
                for h in range(2):
                    m = 2 * j + h
                    for dt in range(8):
                        nc.tensor.matmul(qp[:, h, :], wq[:, m, dt, :],
                                         xt[:, dt, 256:768],
                                         start=(dt == 0), stop=(dt == 7))
                    nc.vector.tensor_copy(qT[m], qp[:, h, :])

            emit_qpair(0)

            def emit_vaug(jts):
                # v_aug: [v | valid*64] (even) / [valid*64 | v] (odd)
                for jt in jts:
                    t_ps = psO.tile([128, 64], BF16, tag="tp", bufs=1)
                    nc.tensor.transpose(t_ps, kv_sb[64:128, ts(jt, 128)],
                                        ident[64:128, 64:128])
                    nc.vector.tensor_copy(vaug_e[:, jt, 0:64], t_ps)
                    nc.vector.tensor_copy(vaug_o[:, jt, 64:128], t_ps)
                    nc.vector.tensor_scalar_mul(vaug_e[:, jt, 64:128],
                                                ones64,
                                                valid_sb[:, jt:jt + 1])
                    nc.vector.tensor_scalar_mul(vaug_o[:, jt, 0:64],
                                                ones64,
                                                valid_sb[:, jt:jt + 1])

            # ---- attention middle ----
            def emit_s_chunk(m, c):
                """Row-tiled S for pair m chunk c: even head on PE rows
                0:63, odd on 64:127, concurrent."""
                s_ps = s_tile(f"sps{m}_{c}")
                pieces, masks = CHUNKS[c]
                for (jt, qlo, off, w, first) in pieces:
                    nc.tensor.matmul(s_ps[:, 0, off:off + w],
                                     kv_sb[0:64, ts(jt, 128)],
                                     qT[m][0:64, qlo:qlo + w],
                                     start=first, stop=True,
                                     skip_group_check=True)
                    nc.tensor.matmul(s_ps[:, 1, off:off + w],
                                     khi[64:128, ts(jt, 128)],
                                     qT[m][64:128, qlo:qlo + w],
                                     start=first, stop=True,
                                     skip_group_check=True)
                ex = expp.tile([128, 2, 512], BF16, tag="ex",
                               name=f"ex{m}_{c}")
                nc.scalar.activation(out=ex, in_=s_ps,
                                     func=AF.Exp, scale=0.125)
                for (doff, lo) in masks:
                    nc.gpsimd.affine_select(
                        out=ex[:, :, doff:doff + 128],
                        in_=ex[:, :, doff:doff + 128],
                        compare_op=ALU.is_ge,
                        fill=0.0, base=0,
                        pattern=[[0, 2], [-1 if lo else 1, 128]],
                        channel_multiplier=1 if lo else -1)
                return ex

            def emit_av_chunk(o_ps, he, va, exs, c):
                pieces, _ = CHUNKS[c]
                for pi, (jt, qlo, off, w, _) in enumerate(pieces):
                    nc.tensor.matmul(
                        o_ps[:, qlo:qlo + w], va[:, jt, :],
                        exs[c][:, he, off:off + w],
                        start=(c == 0 and pi == 0),
                        stop=(c == len(CHUNKS) - 1
                              and pi == len(pieces) - 1),
                        skip_group_check=True)

            def emit_norm(h, o_ps):
                m, r0 = h // 2, 64 * (h % 2)
                odd = h % 2 == 1
                # normalize: denom replicated on the opposite 64 lanes.
                # reciprocal_approx_fast only works on SBUF input at
                # partitions 0:64 on HW, so: DVE-copy the denom out of
                # psum (lane-locked), route via DMA so the recip runs on
                # the lower lanes, multiply on the attn lanes.
                dlo = 0 if odd else 64
                den = recp.tile([128, BLK], F32, tag="den")
                rec = recp.tile([128, BLK], F32, tag="rec")
                nc.vector.tensor_copy(den[dlo:dlo + 64, :],
                                      o_ps[dlo:dlo + 64, :])
                if odd:
                    nc.vector.reciprocal_approx_fast(
                        rec[0:64, :], den[0:64, :])
                    nc.sync.dma_start(out=rec[64:128, :], in_=rec[0:64, :])
                else:
                    nc.sync.dma_start(out=den[0:64, :], in_=den[64:128, :])
                    nc.vector.reciprocal_approx_fast(
                        rec[0:64, :], den[0:64, :])
                nc.vector.tensor_mul(anrmP[m][r0:r0 + 64, :],
                                     o_ps[r0:r0 + 64, :], rec[r0:r0 + 64, :])

            def emit_y1(o):
                # stage-1 Wf chain (ft 0-3) on the psM y bank
                y_ps = psM.tile([128, 512], F32, tag="y", bufs=1,
                                name=f"y1_{o}")
                for ft in range(4):
                    nc.tensor.matmul(y_ps, wf[:, o, ft, :], anrmP[ft],
                                     start=(ft == 0), stop=(ft == 3))
                nc.vector.tensor_copy(y1_sb[o], y_ps)

            prev = None
            for m in range(NH // 2 + 1):
                def avc(c):
                    for he in range(2):
                        emit_av_chunk(prev[1][he], he,
                                      prev[2][he], prev[0], c)
                cur = None
                if m < NH // 2:
                    exs = []
                    if prev is None:
                        # vaug transposes fill the exp-gated gaps
                        # between the first S chunks
                        exs.append(emit_s_chunk(m, 0))
                        exs.append(emit_s_chunk(m, 1))
                        emit_vaug(range(0, 4))
                        exs.append(emit_s_chunk(m, 2))
                        emit_vaug(range(4, 8))
                        exs.append(emit_s_chunk(m, 3))
                        exs.append(emit_s_chunk(m, 4))
                    else:
                        avc(0)
                        avc(1)
                        exs.append(emit_s_chunk(m, 0))
                        avc(2)
                        exs.append(emit_s_chunk(m, 1))
                        avc(3)
                        exs.append(emit_s_chunk(m, 2))
                        avc(4)
                        exs.append(emit_s_chunk(m, 3))
                        exs.append(emit_s_chunk(m, 4))
                    if m <= 2:
                        emit_qpair(m + 1)
                    o_e = psO.tile([128, 512], F32, tag="O",
                                   name=f"o_e{m}")
                    o_o = psO.tile([128, 512], F32, tag="O",
                                   name=f"o_o{m}")
                    cur = (exs, (o_e, o_o), (vaug_e, vaug_o), m)
                else:
                    # last pair: AV first (it gates the final norms),
                    # then the remaining y1 chains
                    for c in range(5):
                        avc(c)
                    emit_y1(6)
                    emit_y1(7)
                if prev is not None:
                    pm = prev[3]
                    emit_norm(2 * pm, prev[1][0])
                    emit_norm(2 * pm + 1, prev[1][1])
                    if 4 <= pm < 7:
                        emit_y1(2 * (pm - 4))
                        emit_y1(2 * (pm - 4) + 1)
                prev = cur

            # ---- tail: y = wf[ft 4:8].T @ anrm + y1 + bf ----
            # ft 4-6 are ungated by the last pair, so every chain
            # pre-runs them; only ft7 + the fused drain wait for the
            # final norm.  Chains sit on psO's ring plus two S-ring
            # tiles (the S banks are free after the last exp).
            y2a = s_tile("y2a")
            y2b = s_tile("y2b")
            slots = [y2a[:, 0, :], y2a[:, 1, :], y2b[:, 0, :], y2b[:, 1, :]]
            y_ps_all = []
            for o in range(8):
                if o < 4:
                    y_ps = slots[o]
                else:
                    y_ps = psO.tile([128, 512], F32, tag="O",
                                    name=f"y2_{o}")
                for ft in range(4, 7):
                    nc.tensor.matmul(y_ps, wf[:, o, ft, :], anrmP[ft],
                                     start=(ft == 4), stop=False,
                                     skip_group_check=True)
                y_ps_all.append(y_ps)
            for o in range(8):
                y_ps = y_ps_all[o]
                nc.tensor.matmul(y_ps, wf[:, o, 7, :], anrmP[7],
                                 start=False, stop=True,
                                 skip_group_check=True)
                y_sb = big.tile([128, BLK], BF16, tag=f"yo{o % 4}",
                                name=f"y_sb{o}")
                nc.vector.scalar_tensor_tensor(
                    out=y_sb, in0=y_ps, scalar=bf_sb[:, o:o + 1],
                    in1=y1_sb[o], op0=ALU.add, op1=ALU.add)
                eng = nc.scalar if o % 2 == 0 else nc.sync
                eng.dma_start(out=yT[ts(o, 128), :], in_=y_sb)

    nc.compile()
    return nc


_NC = None


def _get_nc():
    global _NC
    if _NC is None:
        _NC = _build()
    return _NC


def _swiz(wT):
    """[D, 8blk*128c] row-major -> [p, blk, dt, c] swizzled rows."""
    return np.ascontiguousarray(
        wT.reshape(8, 128, 8, 128).transpose(1, 2, 0, 3).reshape(128, -1))


def _prep_inputs(x, Wq, Wk, Wv, Wf, bf):
    x = np.asarray(x, np.float32)
    wkvT = np.concatenate([np.asarray(Wk, np.float32),
                           np.asarray(Wv, np.float32)], axis=0).T
    shared = {
        # wq/wf: [dt*128+p, m*128+c] -> [p, m*1024 + dt*128 + c]
        "wqT": _swiz(np.asarray(Wq, np.float32).T).astype(BF),
        "wfT": _swiz(np.asarray(Wf, np.float32).T).astype(BF),
        # wkv: [dt*128+p, f] -> [p, dt*128 + f]
        "wkvT": np.ascontiguousarray(
            wkvT.reshape(8, 128, 128).transpose(1, 0, 2).reshape(128, -1)
        ).astype(BF),
        "bfin": np.asarray(bf, np.float32).reshape(D, 1),
    }
    in_maps = []
    for c in range(NCORES):
        b, i = divmod(c, 4)
        g0 = 512 * i - WIN  # global position of ctx col 0
        xTc = np.zeros((D, CTX), np.float32)
        lo, hi = max(0, g0), min(T, g0 + CTX)
        xTc[:, lo - g0:hi - g0] = x[b, lo:hi, :].T
        s = np.arange(CTX)
        vmask = ((s + g0 >= 0) & (s + g0 < T)).astype(np.float32)
        in_maps.append({
            "xT": xTc.astype(BF),
            "valid": np.ascontiguousarray(vmask.reshape(8, 128).T),
            **shared,
        })
    return in_maps


def _run(inputs, trace=False):
    nc = _get_nc()
    in_maps = _prep_inputs(**inputs)
    res = run_bass_kernel_spmd(nc, in_maps, core_ids=list(range(NCORES)),
                               trace=trace)
    x = inputs["x"]
    out = np.empty((B, T, D), np.float32)
    for c in range(NCORES):
        b, i = divmod(c, 4)
        out[b, 512 * i:512 * (i + 1), :] = \
            res.results[c]["yT"].astype(np.float32).T
    return out.astype(np.asarray(x).dtype), res


def kernel(**inputs):
    out, _ = _run(inputs)
    return out
